# revision 19
# baseline (speedup 1.0000x reference)
"""Trainium2 Bass kernel for CustomSpecificity (histogram_binning).

reference:
    t = argmax(y_true, axis=1); p = argmax(y_pred, axis=1)   # [N], C=16
    cm = bincount(t*C+p, length=C*C).reshape(C, C)
    specificity_c = tn_c / (tn_c + fp_c + eps); return mean_c

Strategy (data-parallel over N across 8 cores):
  Each core processes N_CORE = 250k rows of both tensors.
  Per tile [128, R, 16] (rows r contiguous per partition p):
    - rowmax  = reduce_max over class axis            (VectorE, 1 pass)
    - onehot  = (x >= rowmax)  -> bf16 {0,1}          (VectorE, 1 pass)
  The 16x16 confusion matrix is accumulated on TensorE as U^T @ V with
  8 rows packed per partition: lhsT/rhs are [128, 8*16] slabs so each
  matmul covers 1024 rows; only the 8 diagonal 16x16 blocks of the
  [128,128] PSUM tile are meaningful, host extracts + sums them.
  Host sums the per-core partials and applies the tiny specificity
  formula in float32 (exact: all counts are integers < 2^24).

Ties at the row max produce two 1s in a one-hot (reference argmax picks
the first). For the fixed random-normal inputs exact fp32 ties at the
row max are essentially nonexistent (~1e-6 of rows at worst), which
perturbs counts by O(1) out of 2M -> relative error ~1e-6.
"""

import numpy as np

C = 16
N_TOTAL = 2_000_000
N_CORES = 8
N_CORE = N_TOTAL // N_CORES  # 250_000
P = 128

# main-section geometry: N_CORE = P*R*T + N_TAIL, tail handled as a
# [N_TAIL, 16] micro-tile (one row per partition).
R = 279
T = 7
N_MAIN = P * R * T            # 249_984
N_TAIL = N_CORE - N_MAIN      # 16
F = R * C                     # free elems per tensor per tile

_BUILT = {}

# production configuration (HW-validated): VectorE reduce + A-compare,
# GpSimd B-subtract, ScalarE sign for the B one-hot (V' = v-1 form, host
# fixup), ramped first tiles for DMA warm-up, DVE-only last tile.
PROD_TILES = [31, 62, 124] + [217] * 8          # sums to 1953 rows/partition
PROD_KW = dict(gpsimd_split=True, act_sign=True, tile_list=PROD_TILES,
               last_dve=True)
PROD_ACT_SIGN = True


def _build(n_core, r, t, n_tail, gpsimd_split=False, n_rep=1, act_sign=False,
           tile_list=None, last_dve=False):
    """Build + bacc-compile the per-core Bass program.

    gpsimd_split: VectorE computes the combined rowmax reduce and the
    y_true compare; GpSimd computes the y_pred compare concurrently
    (its shared-SBUF-port use overlaps the reduce, which only needs
    DVE's dedicated port).

    n_rep: unrolled repeats of the whole pipeline (for slope timing only;
    output counts scale by n_rep).
    """
    import concourse.bacc as bacc
    import concourse.mybir as mybir
    import concourse.tile as tile

    if tile_list is None:
        tile_list = [r] * t
    assert sum(tile_list) * P + n_tail == n_core
    n_main = P * sum(tile_list)
    # row offsets (per partition-slice) of each tile within the main section
    tile_off = np.cumsum([0] + list(tile_list))[:-1]

    nc = bacc.Bacc("TRN2", target_bir_lowering=False, debug=False)
    f32 = mybir.dt.float32
    bf16 = mybir.dt.bfloat16

    yt = nc.dram_tensor("y_true", [n_core, C], f32, kind="ExternalInput")
    yp = nc.dram_tensor("y_pred", [n_core, C], f32, kind="ExternalInput")
    cm_out = nc.dram_tensor("cm_parts", [P, P], f32, kind="ExternalOutput")

    def main_view(dram, off_r, r_i):
        # tile = rows [off_r*P .. (off_r+r_i)*P) of the main section, laid out
        # partition-contiguous: partition p holds rows off+p*r_i .. off+(p+1)*r_i
        lo = off_r * P
        hi = (off_r + r_i) * P
        return dram[lo:hi].rearrange("(p r) c -> p (r c)", p=P, r=r_i)

    n_mm = (sum((r_i // 8) + (1 if r_i % 8 else 0) for r_i in tile_list)
            + (1 if n_tail else 0)) * n_rep

    with tile.TileContext(nc) as tc:
        with (
            tc.tile_pool(name="ab", bufs=3) as ab_pool,
            tc.tile_pool(name="uv", bufs=2) as uv_pool,
            tc.tile_pool(name="db", bufs=2) as db_pool,
            tc.tile_pool(name="mx", bufs=2) as mx_pool,
            tc.tile_pool(name="tail", bufs=1) as tail_pool,
            tc.tile_pool(name="res", bufs=1) as res_pool,
            tc.tile_pool(name="psum", bufs=1, space="PSUM") as psum_pool,
        ):
            psum = psum_pool.tile([P, P], f32)
            mm_i = 0

            def mm(out_ap, lhsT, rhs):
                nonlocal mm_i
                nc.tensor.matmul(
                    out_ap, lhsT, rhs,
                    start=(mm_i == 0), stop=(mm_i == n_mm - 1),
                    skip_group_check=True,
                )
                mm_i += 1

            for _rep in range(n_rep):
                for ti, r_i in enumerate(tile_list):
                    f_i = r_i * C
                    use_gps = gpsimd_split and not (last_dve and ti == len(tile_list) - 1)
                    ab = ab_pool.tile([P, 2 * f_i], f32, tag="ab")
                    nc.sync.dma_start(out=ab[:, 0:f_i], in_=main_view(yt, tile_off[ti], r_i))
                    nc.sync.dma_start(out=ab[:, f_i:2 * f_i], in_=main_view(yp, tile_off[ti], r_i))

                    ab3 = ab[:].rearrange("p (r c) -> p r c", c=C)
                    mx = mx_pool.tile([P, 2 * r_i], f32, tag="mx")
                    nc.vector.tensor_reduce(
                        mx[:], ab3, axis=mybir.AxisListType.X, op=mybir.AluOpType.max
                    )

                    uv = uv_pool.tile([P, 2 * f_i], bf16, tag="uv")
                    if use_gps:
                        # A-half one-hot on VectorE (TT is_ge, 1x fp32).
                        # B-half: GpSimd subtracts the broadcast rowmax
                        # (sign-exact even after bf16 rounding; only runs
                        # while VectorE is in its reduce, which doesn't use
                        # the shared SBUF port), then either ScalarE sign()
                        # (V' = v-1 form, host fixes up) or a 4x-mode bf16
                        # VectorE tensor_scalar is_ge.
                        a3 = ab[:, 0:f_i].rearrange("p (r c) -> p r c", c=C)
                        b3 = ab[:, f_i:2 * f_i].rearrange("p (r c) -> p r c", c=C)
                        ua3 = uv[:, 0:f_i].rearrange("p (r c) -> p r c", c=C)
                        nc.vector.tensor_tensor(
                            ua3, a3, mx[:, 0:r_i].broadcast_to([P, r_i, C]),
                            op=mybir.AluOpType.is_ge,
                        )
                        db = db_pool.tile([P, f_i], bf16, tag="db")
                        db3 = db[:].rearrange("p (r c) -> p r c", c=C)
                        nc.gpsimd.tensor_tensor(
                            db3, b3, mx[:, r_i:2 * r_i].broadcast_to([P, r_i, C]),
                            op=mybir.AluOpType.subtract,
                        )
                        if act_sign:
                            # V' = sign(B - mx) in {-1, 0}; host fixes up
                            # cm = M - rowsum(M)/15 (exact, linear).
                            nc.scalar.sign(uv[:, f_i:2 * f_i], db[:])
                        else:
                            nc.vector.tensor_single_scalar(
                                uv[:, f_i:2 * f_i], db[:], 0.0, op=mybir.AluOpType.is_ge,
                            )
                    else:
                        uv3 = uv[:].rearrange("p (r c) -> p r c", c=C)
                        nc.vector.tensor_tensor(
                            uv3, ab3, mx[:].broadcast_to([P, 2 * r_i, C]),
                            op=mybir.AluOpType.is_ge,
                        )
                        if act_sign:
                            # match the V' = v - 1 convention of gps tiles
                            nc.vector.tensor_single_scalar(
                                uv[:, f_i:2 * f_i], uv[:, f_i:2 * f_i], -1.0,
                                op=mybir.AluOpType.add,
                            )

                    n_full_i, rem_i = r_i // 8, r_i % 8
                    for g in range(n_full_i):
                        mm(psum[:, :],
                           uv[:, g * P:(g + 1) * P],
                           uv[:, f_i + g * P: f_i + (g + 1) * P])
                    if rem_i:
                        w = rem_i * C
                        mm(psum[0:w, 0:w],
                           uv[:, n_full_i * P: n_full_i * P + w],
                           uv[:, f_i + n_full_i * P: f_i + n_full_i * P + w])

                if n_tail:
                    ta = tail_pool.tile([n_tail, 2 * C], f32, tag="ta")
                    nc.sync.dma_start(out=ta[:, 0:C], in_=yt[n_main:n_core])
                    nc.sync.dma_start(out=ta[:, C:2 * C], in_=yp[n_main:n_core])
                    ta3 = ta[:].rearrange("p (r c) -> p r c", c=C)
                    mt = tail_pool.tile([n_tail, 2], f32, tag="mt")
                    nc.vector.tensor_reduce(
                        mt[:], ta3, axis=mybir.AxisListType.X, op=mybir.AluOpType.max
                    )
                    ut = tail_pool.tile([n_tail, 2 * C], bf16, tag="ut")
                    ut3 = ut[:].rearrange("p (r c) -> p r c", c=C)
                    nc.vector.tensor_tensor(
                        ut3, ta3, mt[:].broadcast_to([n_tail, 2, C]),
                        op=mybir.AluOpType.is_ge,
                    )
                    if act_sign:
                        # match the main tiles' V' = v - 1 convention
                        nc.vector.tensor_single_scalar(
                            ut[:, C:2 * C], ut[:, C:2 * C], -1.0,
                            op=mybir.AluOpType.add,
                        )
                    mm(psum[0:C, 0:C], ut[:, 0:C], ut[:, C:2 * C])

            assert mm_i == n_mm

            # union of PSUM regions actually written by the matmuls
            w_max = (P if any(r_i >= 8 for r_i in tile_list)
                     else max(max(r_i % 8 for r_i in tile_list) * C,
                              C if n_tail else 0))
            res = res_pool.tile([P, P], f32)
            if w_max < P:
                nc.vector.memset(res[:], 0.0)
            nc.vector.tensor_copy(out=res[0:w_max, 0:w_max], in_=psum[0:w_max, 0:w_max])
            nc.sync.dma_start(out=cm_out[:], in_=res[:])

    nc.compile()
    return nc


def _get_nc():
    key = "prod"
    if key not in _BUILT:
        _BUILT[key] = _build(N_CORE, R, T, N_TAIL, **PROD_KW)
    return _BUILT[key]


def _specificity_from_cm(cm, n_total):
    cm = cm.astype(np.float32)
    tp = np.diag(cm).copy()
    fp = cm.sum(axis=0, dtype=np.float32) - tp
    fn = cm.sum(axis=1, dtype=np.float32) - tp
    tn = np.float32(n_total) - (tp + fp + fn)
    eps = np.float32(np.finfo(np.float32).eps)
    spec = tn / (tn + fp + eps)
    return np.array(spec.mean(dtype=np.float32), dtype=np.float32)


_RUNNER = {}


def _get_runner():
    """Cached sharded-jit runner for the axon/PJRT path: one bass_exec per
    jit module, inputs sharded on axis 0 across the 8 cores."""
    if _RUNNER:
        return _RUNNER
    import jax
    from jax.sharding import Mesh, PartitionSpec, NamedSharding
    try:
        from jax.experimental.shard_map import shard_map
    except ImportError:
        from jax.shard_map import shard_map
    import concourse.mybir as mybir
    from concourse import bass2jax
    from concourse.bass2jax import _bass_exec_p, install_neuronx_cc_hook

    install_neuronx_cc_hook()
    nc = _get_nc()
    partition_name = nc.partition_id_tensor.name if nc.partition_id_tensor else None
    in_names, out_names, out_avals = [], [], []
    for alloc in nc.m.functions[0].allocations:
        if not isinstance(alloc, mybir.MemoryLocationSet):
            continue
        name = alloc.memorylocations[0].name
        if alloc.kind == "ExternalInput":
            if name != partition_name:
                in_names.append(name)
        elif alloc.kind == "ExternalOutput":
            out_names.append(name)
            out_avals.append(
                jax.core.ShapedArray(tuple(alloc.tensor_shape), mybir.dt.np(alloc.dtype))
            )
    all_in_names = list(in_names) + list(out_names)
    if partition_name is not None:
        all_in_names.append(partition_name)
    n_params = len(in_names)

    def _body(*args):
        operands = list(args)
        if partition_name is not None:
            operands.append(bass2jax.partition_id_tensor())
        return tuple(_bass_exec_p.bind(
            *operands,
            out_avals=tuple(out_avals),
            in_names=tuple(all_in_names),
            out_names=tuple(out_names),
            lowering_input_output_aliases=(),
            sim_require_finite=True,
            sim_require_nnan=True,
            nc=nc,
        ))

    devices = jax.devices()[:N_CORES]
    mesh = Mesh(np.asarray(devices), ("core",))
    fn = jax.jit(
        shard_map(
            _body, mesh=mesh,
            in_specs=(PartitionSpec("core"),) * (n_params + len(out_names)),
            out_specs=(PartitionSpec("core"),) * len(out_names),
            check_rep=False,
        ),
        donate_argnums=tuple(range(n_params, n_params + len(out_names))),
        keep_unused=True,
    )
    _RUNNER.update(
        fn=fn, in_names=in_names, out_names=out_names, out_avals=out_avals,
        mesh=mesh, sharding=NamedSharding(mesh, PartitionSpec("core")),
        jax=jax,
    )
    return _RUNNER


def _run_axon(y_true, y_pred):
    rn = _get_runner()
    jax = rn["jax"]
    sh = rn["sharding"]
    full = {"y_true": y_true, "y_pred": y_pred}
    dev_in = [jax.device_put(full[name], sh) for name in rn["in_names"]]
    zeros = [
        jax.device_put(np.zeros((N_CORES * av.shape[0], *av.shape[1:]), av.dtype), sh)
        for av in rn["out_avals"]
    ]
    outs = rn["fn"](*dev_in, *zeros)
    av = rn["out_avals"][0]
    return np.asarray(outs[0]).reshape(N_CORES, *av.shape)


def kernel(**inputs):
    y_true = np.ascontiguousarray(np.asarray(inputs["y_true"], dtype=np.float32))
    y_pred = np.ascontiguousarray(np.asarray(inputs["y_pred"], dtype=np.float32))
    assert y_true.shape == (N_TOTAL, C) and y_pred.shape == (N_TOTAL, C)

    from concourse._compat import axon_active
    if axon_active():
        parts = _run_axon(y_true, y_pred)  # [8, 128, 128]
        p_sum = parts.astype(np.float64).sum(axis=0)
    else:
        from concourse.bass_utils import run_bass_kernel_spmd
        nc = _get_nc()
        in_maps = [
            {
                "y_true": y_true[k * N_CORE:(k + 1) * N_CORE],
                "y_pred": y_pred[k * N_CORE:(k + 1) * N_CORE],
            }
            for k in range(N_CORES)
        ]
        res = run_bass_kernel_spmd(nc, in_maps, core_ids=list(range(N_CORES)))
        p_sum = np.zeros((P, P), dtype=np.float64)
        for rmap in res.results:
            p_sum += rmap["cm_parts"].astype(np.float64)

    cm = np.zeros((C, C), dtype=np.float64)
    for s in range(P // C):
        cm += p_sum[C * s:C * (s + 1), C * s:C * (s + 1)]
    if PROD_ACT_SIGN:
        # V' = v - 1 on device; exact linear fixup: cm = M - rowsum(M)/15
        cm = cm - cm.sum(axis=1, keepdims=True) / 15.0
    return _specificity_from_cm(cm, N_TOTAL)


# revision 24
# speedup vs baseline: 1.0323x; 1.0323x over previous
"""Trainium2 Bass kernel for CustomSpecificity (histogram_binning).

reference:
    t = argmax(y_true, axis=1); p = argmax(y_pred, axis=1)   # [N], C=16
    cm = bincount(t*C+p, length=C*C).reshape(C, C)
    specificity_c = tn_c / (tn_c + fp_c + eps); return mean_c

Strategy (data-parallel over N across 8 cores):
  Each core processes N_CORE = 250k rows of both tensors.
  Per tile [128, R, 16] (rows r contiguous per partition p):
    - rowmax  = reduce_max over class axis            (VectorE, 1 pass)
    - onehot  = (x >= rowmax)  -> bf16 {0,1}          (VectorE, 1 pass)
  The 16x16 confusion matrix is accumulated on TensorE as U^T @ V with
  8 rows packed per partition: lhsT/rhs are [128, 8*16] slabs so each
  matmul covers 1024 rows; only the 8 diagonal 16x16 blocks of the
  [128,128] PSUM tile are meaningful, host extracts + sums them.
  Host sums the per-core partials and applies the tiny specificity
  formula in float32 (exact: all counts are integers < 2^24).

Ties at the row max produce two 1s in a one-hot (reference argmax picks
the first). For the fixed random-normal inputs exact fp32 ties at the
row max are essentially nonexistent (~1e-6 of rows at worst), which
perturbs counts by O(1) out of 2M -> relative error ~1e-6.
"""

import numpy as np

C = 16
N_TOTAL = 2_000_000
N_CORES = 8
N_CORE = N_TOTAL // N_CORES  # 250_000
P = 128

# main-section geometry: N_CORE = P*R*T + N_TAIL, tail handled as a
# [N_TAIL, 16] micro-tile (one row per partition).
R = 279
T = 7
N_MAIN = P * R * T            # 249_984
N_TAIL = N_CORE - N_MAIN      # 16
F = R * C                     # free elems per tensor per tile

_BUILT = {}

# production configuration (HW-validated): VectorE reduce + A-compare,
# GpSimd B-subtract, ScalarE sign for the B one-hot (V' = v-1 form, host
# fixup), ramped first tiles for DMA warm-up, DVE-only last tile.
PROD_TILES = [31, 62, 124] + [217] * 7 + [186, 31]   # sums to 1953 rows/partition
PROD_KW = dict(gpsimd_split=True, act_sign=True, tile_list=PROD_TILES,
               last_dve=1)
PROD_ACT_SIGN = True


def _build(n_core, r, t, n_tail, gpsimd_split=False, n_rep=1, act_sign=False,
           tile_list=None, last_dve=False):
    """Build + bacc-compile the per-core Bass program.

    gpsimd_split: VectorE computes the combined rowmax reduce and the
    y_true compare; GpSimd computes the y_pred compare concurrently
    (its shared-SBUF-port use overlaps the reduce, which only needs
    DVE's dedicated port).

    n_rep: unrolled repeats of the whole pipeline (for slope timing only;
    output counts scale by n_rep).
    """
    import concourse.bacc as bacc
    import concourse.mybir as mybir
    import concourse.tile as tile

    if tile_list is None:
        tile_list = [r] * t
    assert sum(tile_list) * P + n_tail == n_core
    n_main = P * sum(tile_list)
    # row offsets (per partition-slice) of each tile within the main section
    tile_off = np.cumsum([0] + list(tile_list))[:-1]

    nc = bacc.Bacc("TRN2", target_bir_lowering=False, debug=False)
    f32 = mybir.dt.float32
    bf16 = mybir.dt.bfloat16

    yt = nc.dram_tensor("y_true", [n_core, C], f32, kind="ExternalInput")
    yp = nc.dram_tensor("y_pred", [n_core, C], f32, kind="ExternalInput")
    cm_out = nc.dram_tensor("cm_parts", [P, P], f32, kind="ExternalOutput")

    def main_view(dram, off_r, r_i):
        # tile = rows [off_r*P .. (off_r+r_i)*P) of the main section, laid out
        # partition-contiguous: partition p holds rows off+p*r_i .. off+(p+1)*r_i
        lo = off_r * P
        hi = (off_r + r_i) * P
        return dram[lo:hi].rearrange("(p r) c -> p (r c)", p=P, r=r_i)

    n_mm = (sum((r_i // 8) + (1 if r_i % 8 else 0) for r_i in tile_list)
            + (1 if n_tail else 0)) * n_rep

    with tile.TileContext(nc) as tc:
        with (
            tc.tile_pool(name="ab", bufs=3) as ab_pool,
            tc.tile_pool(name="uv", bufs=2) as uv_pool,
            tc.tile_pool(name="db", bufs=2) as db_pool,
            tc.tile_pool(name="mx", bufs=2) as mx_pool,
            tc.tile_pool(name="tail", bufs=1) as tail_pool,
            tc.tile_pool(name="res", bufs=1) as res_pool,
            tc.tile_pool(name="psum", bufs=1, space="PSUM") as psum_pool,
        ):
            psum = psum_pool.tile([P, P], f32)
            mm_i = 0

            def mm(out_ap, lhsT, rhs):
                nonlocal mm_i
                nc.tensor.matmul(
                    out_ap, lhsT, rhs,
                    start=(mm_i == 0), stop=(mm_i == n_mm - 1),
                    skip_group_check=True,
                )
                mm_i += 1

            for _rep in range(n_rep):
                for ti, r_i in enumerate(tile_list):
                    f_i = r_i * C
                    n_last_dve = int(last_dve)  # count of trailing DVE-only tiles
                    use_gps = gpsimd_split and ti < len(tile_list) - n_last_dve
                    ab = ab_pool.tile([P, 2 * f_i], f32, tag="ab")
                    nc.sync.dma_start(out=ab[:, 0:f_i], in_=main_view(yt, tile_off[ti], r_i))
                    nc.sync.dma_start(out=ab[:, f_i:2 * f_i], in_=main_view(yp, tile_off[ti], r_i))

                    ab3 = ab[:].rearrange("p (r c) -> p r c", c=C)
                    mx = mx_pool.tile([P, 2 * r_i], f32, tag="mx")
                    nc.vector.tensor_reduce(
                        mx[:], ab3, axis=mybir.AxisListType.X, op=mybir.AluOpType.max
                    )

                    uv = uv_pool.tile([P, 2 * f_i], bf16, tag="uv")
                    if use_gps:
                        # A-half one-hot on VectorE (TT is_ge, 1x fp32).
                        # B-half: GpSimd subtracts the broadcast rowmax
                        # (sign-exact even after bf16 rounding; only runs
                        # while VectorE is in its reduce, which doesn't use
                        # the shared SBUF port), then either ScalarE sign()
                        # (V' = v-1 form, host fixes up) or a 4x-mode bf16
                        # VectorE tensor_scalar is_ge.
                        a3 = ab[:, 0:f_i].rearrange("p (r c) -> p r c", c=C)
                        b3 = ab[:, f_i:2 * f_i].rearrange("p (r c) -> p r c", c=C)
                        ua3 = uv[:, 0:f_i].rearrange("p (r c) -> p r c", c=C)
                        nc.vector.tensor_tensor(
                            ua3, a3, mx[:, 0:r_i].broadcast_to([P, r_i, C]),
                            op=mybir.AluOpType.is_ge,
                        )
                        db = db_pool.tile([P, f_i], bf16, tag="db")
                        db3 = db[:].rearrange("p (r c) -> p r c", c=C)
                        nc.gpsimd.tensor_tensor(
                            db3, b3, mx[:, r_i:2 * r_i].broadcast_to([P, r_i, C]),
                            op=mybir.AluOpType.subtract,
                        )
                        if act_sign:
                            # V' = sign(B - mx) in {-1, 0}; host fixes up
                            # cm = M - rowsum(M)/15 (exact, linear).
                            nc.scalar.sign(uv[:, f_i:2 * f_i], db[:])
                        else:
                            nc.vector.tensor_single_scalar(
                                uv[:, f_i:2 * f_i], db[:], 0.0, op=mybir.AluOpType.is_ge,
                            )
                    else:
                        uv3 = uv[:].rearrange("p (r c) -> p r c", c=C)
                        nc.vector.tensor_tensor(
                            uv3, ab3, mx[:].broadcast_to([P, 2 * r_i, C]),
                            op=mybir.AluOpType.is_ge,
                        )
                        if act_sign:
                            # match the V' = v - 1 convention of gps tiles
                            nc.vector.tensor_single_scalar(
                                uv[:, f_i:2 * f_i], uv[:, f_i:2 * f_i], -1.0,
                                op=mybir.AluOpType.add,
                            )

                    n_full_i, rem_i = r_i // 8, r_i % 8
                    for g in range(n_full_i):
                        mm(psum[:, :],
                           uv[:, g * P:(g + 1) * P],
                           uv[:, f_i + g * P: f_i + (g + 1) * P])
                    if rem_i:
                        w = rem_i * C
                        mm(psum[0:w, 0:w],
                           uv[:, n_full_i * P: n_full_i * P + w],
                           uv[:, f_i + n_full_i * P: f_i + n_full_i * P + w])

                    if ti == 0 and n_tail:
                        # tail micro-tile emitted early so its serial chain
                        # hides under the main pipeline instead of at the end
                        ta = tail_pool.tile([n_tail, 2 * C], f32, tag="ta")
                        nc.sync.dma_start(out=ta[:, 0:C], in_=yt[n_main:n_core])
                        nc.sync.dma_start(out=ta[:, C:2 * C], in_=yp[n_main:n_core])
                        ta3 = ta[:].rearrange("p (r c) -> p r c", c=C)
                        mt = tail_pool.tile([n_tail, 2], f32, tag="mt")
                        nc.vector.tensor_reduce(
                            mt[:], ta3, axis=mybir.AxisListType.X,
                            op=mybir.AluOpType.max,
                        )
                        ut = tail_pool.tile([n_tail, 2 * C], bf16, tag="ut")
                        ut3 = ut[:].rearrange("p (r c) -> p r c", c=C)
                        nc.vector.tensor_tensor(
                            ut3, ta3, mt[:].broadcast_to([n_tail, 2, C]),
                            op=mybir.AluOpType.is_ge,
                        )
                        if act_sign:
                            # match the main tiles' V' = v - 1 convention
                            nc.vector.tensor_single_scalar(
                                ut[:, C:2 * C], ut[:, C:2 * C], -1.0,
                                op=mybir.AluOpType.add,
                            )
                        mm(psum[0:C, 0:C], ut[:, 0:C], ut[:, C:2 * C])

                pass  # tail handled inline after the first tile (see below)

            assert mm_i == n_mm

            # union of PSUM regions actually written by the matmuls
            w_max = (P if any(r_i >= 8 for r_i in tile_list)
                     else max(max(r_i % 8 for r_i in tile_list) * C,
                              C if n_tail else 0))
            res = res_pool.tile([P, P], f32)
            if w_max < P:
                nc.vector.memset(res[:], 0.0)
            nc.vector.tensor_copy(out=res[0:w_max, 0:w_max], in_=psum[0:w_max, 0:w_max])
            nc.sync.dma_start(out=cm_out[:], in_=res[:])

    nc.compile()
    return nc


def _get_nc():
    key = "prod"
    if key not in _BUILT:
        _BUILT[key] = _build(N_CORE, R, T, N_TAIL, **PROD_KW)
    return _BUILT[key]


def _specificity_from_cm(cm, n_total):
    cm = cm.astype(np.float32)
    tp = np.diag(cm).copy()
    fp = cm.sum(axis=0, dtype=np.float32) - tp
    fn = cm.sum(axis=1, dtype=np.float32) - tp
    tn = np.float32(n_total) - (tp + fp + fn)
    eps = np.float32(np.finfo(np.float32).eps)
    spec = tn / (tn + fp + eps)
    return np.array(spec.mean(dtype=np.float32), dtype=np.float32)


_RUNNER = {}


def _get_runner():
    """Cached sharded-jit runner for the axon/PJRT path: one bass_exec per
    jit module, inputs sharded on axis 0 across the 8 cores."""
    if _RUNNER:
        return _RUNNER
    import jax
    from jax.sharding import Mesh, PartitionSpec, NamedSharding
    try:
        from jax.experimental.shard_map import shard_map
    except ImportError:
        from jax.shard_map import shard_map
    import concourse.mybir as mybir
    from concourse import bass2jax
    from concourse.bass2jax import _bass_exec_p, install_neuronx_cc_hook

    install_neuronx_cc_hook()
    nc = _get_nc()
    partition_name = nc.partition_id_tensor.name if nc.partition_id_tensor else None
    in_names, out_names, out_avals = [], [], []
    for alloc in nc.m.functions[0].allocations:
        if not isinstance(alloc, mybir.MemoryLocationSet):
            continue
        name = alloc.memorylocations[0].name
        if alloc.kind == "ExternalInput":
            if name != partition_name:
                in_names.append(name)
        elif alloc.kind == "ExternalOutput":
            out_names.append(name)
            out_avals.append(
                jax.core.ShapedArray(tuple(alloc.tensor_shape), mybir.dt.np(alloc.dtype))
            )
    all_in_names = list(in_names) + list(out_names)
    if partition_name is not None:
        all_in_names.append(partition_name)
    n_params = len(in_names)

    def _body(*args):
        operands = list(args)
        if partition_name is not None:
            operands.append(bass2jax.partition_id_tensor())
        return tuple(_bass_exec_p.bind(
            *operands,
            out_avals=tuple(out_avals),
            in_names=tuple(all_in_names),
            out_names=tuple(out_names),
            lowering_input_output_aliases=(),
            sim_require_finite=True,
            sim_require_nnan=True,
            nc=nc,
        ))

    devices = jax.devices()[:N_CORES]
    mesh = Mesh(np.asarray(devices), ("core",))
    fn = jax.jit(
        shard_map(
            _body, mesh=mesh,
            in_specs=(PartitionSpec("core"),) * (n_params + len(out_names)),
            out_specs=(PartitionSpec("core"),) * len(out_names),
            check_rep=False,
        ),
        donate_argnums=tuple(range(n_params, n_params + len(out_names))),
        keep_unused=True,
    )
    _RUNNER.update(
        fn=fn, in_names=in_names, out_names=out_names, out_avals=out_avals,
        mesh=mesh, sharding=NamedSharding(mesh, PartitionSpec("core")),
        jax=jax,
    )
    return _RUNNER


def _run_axon(y_true, y_pred):
    rn = _get_runner()
    jax = rn["jax"]
    sh = rn["sharding"]
    full = {"y_true": y_true, "y_pred": y_pred}
    dev_in = [jax.device_put(full[name], sh) for name in rn["in_names"]]
    zeros = [
        jax.device_put(np.zeros((N_CORES * av.shape[0], *av.shape[1:]), av.dtype), sh)
        for av in rn["out_avals"]
    ]
    outs = rn["fn"](*dev_in, *zeros)
    av = rn["out_avals"][0]
    return np.asarray(outs[0]).reshape(N_CORES, *av.shape)


def kernel(**inputs):
    y_true = np.ascontiguousarray(np.asarray(inputs["y_true"], dtype=np.float32))
    y_pred = np.ascontiguousarray(np.asarray(inputs["y_pred"], dtype=np.float32))
    assert y_true.shape == (N_TOTAL, C) and y_pred.shape == (N_TOTAL, C)

    from concourse._compat import axon_active
    if axon_active():
        parts = _run_axon(y_true, y_pred)  # [8, 128, 128]
        p_sum = parts.astype(np.float64).sum(axis=0)
    else:
        from concourse.bass_utils import run_bass_kernel_spmd
        nc = _get_nc()
        in_maps = [
            {
                "y_true": y_true[k * N_CORE:(k + 1) * N_CORE],
                "y_pred": y_pred[k * N_CORE:(k + 1) * N_CORE],
            }
            for k in range(N_CORES)
        ]
        res = run_bass_kernel_spmd(nc, in_maps, core_ids=list(range(N_CORES)))
        p_sum = np.zeros((P, P), dtype=np.float64)
        for rmap in res.results:
            p_sum += rmap["cm_parts"].astype(np.float64)

    cm = np.zeros((C, C), dtype=np.float64)
    for s in range(P // C):
        cm += p_sum[C * s:C * (s + 1), C * s:C * (s + 1)]
    if PROD_ACT_SIGN:
        # V' = v - 1 on device; exact linear fixup: cm = M - rowsum(M)/15
        cm = cm - cm.sum(axis=1, keepdims=True) / 15.0
    return _specificity_from_cm(cm, N_TOTAL)
